# revision 5
# baseline (speedup 1.0000x reference)
"""Trainium2 Bass kernel for nn_AlltagRandomGenerator (scatter_memory).

Reference computation per token (B=512, S=1024 tokens):
    pm   = priv_pos[pos]
    obf  = pm ? (pri_rand < 1.0) : (ctx_rand < 0.15)
    cnt  = counts[pos]
    idx  = min(int(cand_u * float(cnt)), cnt - 1)
    cdt  = tgt_table[pos, idx]
    obf_word = obf ? cdt : word
    obf_char = lut[obf_word]            # [32] chars
    cpy_mask = inp_mask & (word == obf_word)

Kernel strategy (pure data-parallel over batch across 8 cores):
  * Host precomputes a fused row table big[260000, 33] int32:
      rows [p*4000 + i]  = [tgt_table[p,i], lut[tgt_table[p,i]][0:32]]
      rows [160000 + w]  = [w,              lut[w][0:32]]
    so ONE gathered row yields both obf_word and obf_char, collapsing the
    reference's two dependent gathers (tgt_table then lut) into one.
  * Host packs the two 40-entry tables into etab[40] = priv*16384 + counts.
  * Device: gpsimd ap_gather for the per-token 40-entry lookup (indices are
    shared per 16-partition group, so output is 16x expanded; a mask-multiply
    + inner-16 reduce extracts the per-partition diagonal), vector-engine
    index math, then a single gpsimd indirect DMA gather of 132B rows per
    token chunk. A 2-byte-dtype strided copy (DVE 4x mode) strips the word-id
    column so obf_char DMA-out descriptors are large and contiguous.
"""

import numpy as np

import concourse.bass as bass
import concourse.bacc as bacc
import concourse.tile as tile
from concourse import mybir
from concourse.bass_utils import run_bass_kernel_spmd

# Problem constants (hardcoded per harness contract).
B, S = 512, 1024
VOCAB = 100000
CHAR_LEN = 32
N_POS = 40
MAX_CANDS = 4000
CTX_RATE = 0.15
PRI_RATE = 1.0
N_CORES = 8

P = 128                      # partitions
BS_CORE = (B // N_CORES) * S     # tokens per core = 65536
F = BS_CORE // P             # free dim per partition = 512
N_CAND = N_POS * MAX_CANDS   # 160000
N_ROWS = N_CAND + VOCAB      # 260000
ROW = CHAR_LEN + 1           # 33 ints per row: [word_id, chars...]
GCHUNK = 128                 # tokens-per-partition per indirect-gather chunk
N_GC = F // GCHUNK           # gather chunks
ECHUNK = 128                 # cols per ap_gather chunk
N_EC = F // ECHUNK

i32 = mybir.dt.int32
i16 = mybir.dt.int16
u8 = mybir.dt.uint8
f32 = mybir.dt.float32
Alu = mybir.AluOpType

_CACHE = {}


def _build_nc():
    nc = bacc.Bacc("TRN2", target_bir_lowering=False, debug=False)

    word = nc.declare_dram_parameter("word", [P, F], i32, isOutput=False)
    pos = nc.declare_dram_parameter("pos", [P, F], i32, isOutput=False)
    msk = nc.declare_dram_parameter("msk", [P, F], i32, isOutput=False)
    ctx = nc.declare_dram_parameter("ctx", [P, F], f32, isOutput=False)
    pri = nc.declare_dram_parameter("pri", [P, F], f32, isOutput=False)
    cu = nc.declare_dram_parameter("cu", [P, F], f32, isOutput=False)
    btab = nc.declare_dram_parameter("btab", [N_ROWS, ROW], i32, isOutput=False)
    etab = nc.declare_dram_parameter("etab", [N_POS], i32, isOutput=False)
    m16 = nc.declare_dram_parameter("m16", [P, 16], i32, isOutput=False)

    o_word = nc.declare_dram_parameter("o_word", [P, F], i32, isOutput=True)
    o_char = nc.declare_dram_parameter("o_char", [P, F, CHAR_LEN], i32, isOutput=True)
    o_obf = nc.declare_dram_parameter("o_obf", [P, F], u8, isOutput=True)
    o_pri = nc.declare_dram_parameter("o_pri", [P, F], u8, isOutput=True)
    o_cpy = nc.declare_dram_parameter("o_cpy", [P, F], u8, isOutput=True)

    with tile.TileContext(nc) as tc:
        with (
            tc.tile_pool(name="singles", bufs=1) as singles,
            tc.tile_pool(name="inp", bufs=1) as inp,
            tc.tile_pool(name="ew", bufs=1) as ew,
            tc.tile_pool(name="exp", bufs=2) as expp,
            tc.tile_pool(name="grows", bufs=2) as growsp,
            tc.tile_pool(name="cc", bufs=2) as ccp,
        ):
            # --- constants ---
            etab_sb = singles.tile([P, N_POS], i32)
            eap = etab[:]
            nc.sync.dma_start(
                out=etab_sb[:, :],
                in_=bass.AP(
                    tensor=eap.tensor, offset=eap.offset, ap=[[0, P], eap.ap[0]]
                ),
            )
            m16_sb = singles.tile([P, 16], i32)
            nc.sync.dma_start(out=m16_sb[:, :], in_=m16[:, :])

            # --- load per-token inputs ---
            t_word = inp.tile([P, F], i32)
            t_pos = inp.tile([P, F], i32)
            t_msk = inp.tile([P, F], i32)
            t_ctx = inp.tile([P, F], f32)
            t_pri = inp.tile([P, F], f32)
            t_cu = inp.tile([P, F], f32)
            nc.sync.dma_start(out=t_word[:, :], in_=word[:, :])
            nc.sync.dma_start(out=t_pos[:, :], in_=pos[:, :])
            nc.sync.dma_start(out=t_msk[:, :], in_=msk[:, :])
            nc.sync.dma_start(out=t_ctx[:, :], in_=ctx[:, :])
            nc.sync.dma_start(out=t_pri[:, :], in_=pri[:, :])
            nc.sync.dma_start(out=t_cu[:, :], in_=cu[:, :])

            # --- 40-entry table lookup: E[p, j] = etab[pos[p, j]] ---
            pos16 = ew.tile([P, F], i16)
            nc.vector.tensor_copy(out=pos16[:, :], in_=t_pos[:, :])

            t_E = ew.tile([P, F], i32)
            for c in range(N_EC):
                cs = c * ECHUNK
                ce = cs + ECHUNK
                nidx = 16 * ECHUNK
                t_exp = expp.tile([P, nidx], i32)
                nc.gpsimd.ap_gather(
                    out_ap=t_exp[:, :],
                    in_ap=etab_sb[:, :],
                    idxs_ap=pos16[:, cs:ce],
                    channels=P,
                    num_elems=N_POS,
                    d=1,
                    num_idxs=nidx,
                )
                # mask out all but the per-partition diagonal, reduce inner 16
                exp3 = t_exp[:, :].rearrange("p (j r) -> p j r", r=16)
                m16b = bass.AP(
                    tensor=m16_sb.tensor,
                    offset=m16_sb[:, :].offset,
                    ap=[m16_sb[:, :].ap[0], [0, ECHUNK], m16_sb[:, :].ap[1]],
                )
                t_prod = expp.tile([P, nidx], i32, tag="prod")
                nc.vector.tensor_tensor(
                    out=t_prod[:, :].rearrange("p (j r) -> p j r", r=16),
                    in0=exp3,
                    in1=m16b,
                    op=Alu.mult,
                )
                with nc.allow_low_precision(
                    reason="masked one-hot sum of values < 2^15; exact in f32"
                ):
                    nc.vector.tensor_reduce(
                        out=t_E[:, cs:ce],
                        in_=t_prod[:, :].rearrange("p (j r) -> p j r", r=16),
                        axis=mybir.AxisListType.X,
                        op=Alu.add,
                    )

            # --- elementwise index math (full [P, F] tiles) ---
            # etab value E = priv*16384 + counts
            t_cnt = ew.tile([P, F], i32)
            nc.vector.tensor_scalar(
                out=t_cnt[:, :], in0=t_E[:, :], scalar1=16383, scalar2=None,
                op0=Alu.bitwise_and,
            )
            t_cntf = ew.tile([P, F], f32)
            nc.vector.tensor_copy(out=t_cntf[:, :], in_=t_cnt[:, :])
            t_cm1 = ew.tile([P, F], i32)
            nc.vector.tensor_scalar(
                out=t_cm1[:, :], in0=t_cnt[:, :], scalar1=-1, scalar2=None,
                op0=Alu.add,
            )
            t_pm = ew.tile([P, F], u8)
            nc.vector.tensor_scalar(
                out=t_pm[:, :], in0=t_E[:, :], scalar1=16383, scalar2=None,
                op0=Alu.is_gt,
            )

            t_t = ew.tile([P, F], f32)
            nc.vector.tensor_tensor(
                out=t_t[:, :], in0=t_cu[:, :], in1=t_cntf[:, :], op=Alu.mult
            )
            t_it = ew.tile([P, F], i32)
            nc.vector.tensor_copy(out=t_it[:, :], in_=t_t[:, :])
            t_tf = ew.tile([P, F], f32)
            nc.vector.tensor_copy(out=t_tf[:, :], in_=t_it[:, :])
            t_gt = ew.tile([P, F], i32)
            nc.vector.tensor_tensor(
                out=t_gt[:, :], in0=t_tf[:, :], in1=t_t[:, :], op=Alu.is_gt
            )
            t_it2 = ew.tile([P, F], i32)
            nc.vector.tensor_tensor(
                out=t_it2[:, :], in0=t_it[:, :], in1=t_gt[:, :], op=Alu.subtract
            )
            t_idx = ew.tile([P, F], i32)
            nc.vector.tensor_tensor(
                out=t_idx[:, :], in0=t_it2[:, :], in1=t_cm1[:, :], op=Alu.min
            )
            # gobf = pos*4000 + idx  (candidate row)
            t_gobf = ew.tile([P, F], i32)
            nc.vector.scalar_tensor_tensor(
                out=t_gobf[:, :], in0=t_pos[:, :], scalar=MAX_CANDS,
                in1=t_idx[:, :], op0=Alu.mult, op1=Alu.add,
            )
            # g = word + 160000 (identity row), overwritten where obf
            t_g = ew.tile([P, F], i32)
            nc.vector.tensor_scalar(
                out=t_g[:, :], in0=t_word[:, :], scalar1=N_CAND, scalar2=None,
                op0=Alu.add,
            )
            # obf mask
            t_a = ew.tile([P, F], u8)
            nc.vector.tensor_scalar(
                out=t_a[:, :], in0=t_pri[:, :], scalar1=PRI_RATE, scalar2=None,
                op0=Alu.is_lt,
            )
            t_obf = ew.tile([P, F], u8)
            nc.vector.tensor_scalar(
                out=t_obf[:, :], in0=t_ctx[:, :], scalar1=CTX_RATE, scalar2=None,
                op0=Alu.is_lt,
            )
            nc.vector.copy_predicated(
                out=t_obf[:, :], mask=t_pm[:, :], data=t_a[:, :]
            )
            nc.vector.copy_predicated(
                out=t_g[:, :], mask=t_obf[:, :], data=t_gobf[:, :]
            )

            # --- big gather: rows of 33 ints -> w column + char block ---
            # HW indirect DMA consumes exactly one offset per partition per
            # call (multi-index offset APs are broken in walrus), so gather
            # one 128-token column at a time.
            t_w = ew.tile([P, F], i32)
            for c in range(N_GC):
                cs = c * GCHUNK
                ce = cs + GCHUNK
                t_rows = growsp.tile([P, GCHUNK, ROW], i32)
                for k in range(GCHUNK):
                    nc.gpsimd.indirect_dma_start(
                        out=t_rows[:, k, :],
                        out_offset=None,
                        in_=btab[:, :],
                        in_offset=bass.IndirectOffsetOnAxis(
                            ap=t_g[:, cs + k : cs + k + 1], axis=0
                        ),
                    )
                # word-id column (stride ROW)
                nc.vector.tensor_copy(
                    out=t_w[:, cs:ce], in_=t_rows[:, :, 0]
                )
                # strip the id column: copy chars as packed int16 (DVE 4x)
                rows16 = t_rows[:, :, :].bitcast(i16)  # [P, GCHUNK, 2*ROW]
                t_cc = ccp.tile([P, GCHUNK * CHAR_LEN * 2], i16)
                nc.vector.tensor_copy(
                    out=t_cc[:, :].rearrange(
                        "p (j k) -> p j k", k=2 * CHAR_LEN
                    ),
                    in_=rows16[:, :, 2 : 2 + 2 * CHAR_LEN],
                )
                nc.sync.dma_start(
                    out=o_char[:, cs:ce, :],
                    in_=t_cc[:, :].bitcast(i32),
                )

            # --- epilogue masks + small outputs ---
            t_weq = ew.tile([P, F], u8)
            nc.vector.tensor_tensor(
                out=t_weq[:, :], in0=t_w[:, :], in1=t_word[:, :], op=Alu.is_equal
            )
            t_m8 = ew.tile([P, F], u8)
            nc.vector.tensor_copy(out=t_m8[:, :], in_=t_msk[:, :])
            t_cpy = ew.tile([P, F], u8)
            nc.vector.tensor_tensor(
                out=t_cpy[:, :], in0=t_weq[:, :], in1=t_m8[:, :],
                op=Alu.logical_and,
            )

            nc.sync.dma_start(out=o_word[:, :], in_=t_w[:, :])
            nc.sync.dma_start(out=o_obf[:, :], in_=t_obf[:, :])
            nc.sync.dma_start(out=o_pri[:, :], in_=t_pm[:, :])
            nc.sync.dma_start(out=o_cpy[:, :], in_=t_cpy[:, :])

    nc.finalize()
    return nc


def _build_tables(lut, tgt_table, counts, priv_pos):
    lut = np.asarray(lut, dtype=np.int32)
    tgt = np.asarray(tgt_table, dtype=np.int32)
    counts = np.asarray(counts, dtype=np.int32)
    priv = np.asarray(priv_pos).astype(bool)

    btab = np.empty((N_ROWS, ROW), dtype=np.int32)
    tflat = tgt.reshape(-1)
    btab[:N_CAND, 0] = tflat
    btab[:N_CAND, 1:] = lut[tflat]
    btab[N_CAND:, 0] = np.arange(VOCAB, dtype=np.int32)
    btab[N_CAND:, 1:] = lut

    etab = (counts + (priv.astype(np.int32) << 14)).astype(np.int32)

    m16 = (np.arange(16)[None, :] == (np.arange(P) % 16)[:, None]).astype(
        np.int32
    )
    return btab, etab, m16


def kernel(
    inp_word,
    inp_char,
    inp_pos,
    inp_mask,
    ctx_rand,
    pri_rand,
    cand_u,
    lut,
    tgt_table,
    counts,
    priv_pos,
):
    if "nc" not in _CACHE:
        _CACHE["nc"] = _build_nc()
    nc = _CACHE["nc"]

    btab, etab, m16 = _build_tables(lut, tgt_table, counts, priv_pos)

    inp_word = np.ascontiguousarray(np.asarray(inp_word, dtype=np.int32))
    inp_pos = np.ascontiguousarray(np.asarray(inp_pos, dtype=np.int32))
    inp_mask = np.ascontiguousarray(np.asarray(inp_mask, dtype=np.int32))
    ctx_rand = np.ascontiguousarray(np.asarray(ctx_rand, dtype=np.float32))
    pri_rand = np.ascontiguousarray(np.asarray(pri_rand, dtype=np.float32))
    cand_u = np.ascontiguousarray(np.asarray(cand_u, dtype=np.float32))

    bpc = B // N_CORES
    in_maps = []
    for i in range(N_CORES):
        sl = slice(i * bpc, (i + 1) * bpc)
        in_maps.append(
            {
                "word": inp_word[sl].reshape(P, F),
                "pos": inp_pos[sl].reshape(P, F),
                "msk": inp_mask[sl].reshape(P, F),
                "ctx": ctx_rand[sl].reshape(P, F),
                "pri": pri_rand[sl].reshape(P, F),
                "cu": cand_u[sl].reshape(P, F),
                "btab": btab,
                "etab": etab,
                "m16": m16,
            }
        )

    res = run_bass_kernel_spmd(nc, in_maps, core_ids=list(range(N_CORES)))
    _CACHE["last_result"] = res

    obf_word = np.empty((B, S), dtype=np.int32)
    obf_char = np.empty((B, S, CHAR_LEN), dtype=np.int32)
    obf_mask = np.empty((B, S), dtype=bool)
    pri_mask = np.empty((B, S), dtype=bool)
    cpy_mask = np.empty((B, S), dtype=bool)
    for i in range(N_CORES):
        sl = slice(i * bpc, (i + 1) * bpc)
        r = res.results[i]
        obf_word[sl] = r["o_word"].reshape(bpc, S)
        obf_char[sl] = r["o_char"].reshape(bpc, S, CHAR_LEN)
        obf_mask[sl] = r["o_obf"].reshape(bpc, S).astype(bool)
        pri_mask[sl] = r["o_pri"].reshape(bpc, S).astype(bool)
        cpy_mask[sl] = r["o_cpy"].reshape(bpc, S).astype(bool)

    return (obf_word, inp_word, obf_char, inp_pos, obf_mask, pri_mask, cpy_mask)


# revision 6
# speedup vs baseline: 1.0029x; 1.0029x over previous
"""Trainium2 Bass kernel for nn_AlltagRandomGenerator (scatter_memory).

Reference computation per token (B=512, S=1024 tokens):
    pm   = priv_pos[pos]
    obf  = pm ? (pri_rand < 1.0) : (ctx_rand < 0.15)
    cnt  = counts[pos]
    idx  = min(int(cand_u * float(cnt)), cnt - 1)
    cdt  = tgt_table[pos, idx]
    obf_word = obf ? cdt : word
    obf_char = lut[obf_word]            # [32] chars
    cpy_mask = inp_mask & (word == obf_word)

Kernel strategy (pure data-parallel over batch across 8 cores):
  * Host precomputes a fused row table big[260000, 33] int32:
      rows [p*4000 + i]  = [tgt_table[p,i], lut[tgt_table[p,i]][0:32]]
      rows [160000 + w]  = [w,              lut[w][0:32]]
    so ONE gathered row yields both obf_word and obf_char, collapsing the
    reference's two dependent gathers (tgt_table then lut) into one.
  * Host packs the two 40-entry tables into etab[40] = priv*16384 + counts.
  * Device: gpsimd ap_gather for the per-token 40-entry lookup (indices are
    shared per 16-partition group, so output is 16x expanded; a mask-multiply
    + inner-16 reduce extracts the per-partition diagonal), vector-engine
    index math, then a single gpsimd indirect DMA gather of 132B rows per
    token chunk. A 2-byte-dtype strided copy (DVE 4x mode) strips the word-id
    column so obf_char DMA-out descriptors are large and contiguous.
"""

import numpy as np

import concourse.bass as bass
import concourse.bacc as bacc
import concourse.tile as tile
from concourse import mybir
from concourse.bass_utils import run_bass_kernel_spmd

# Problem constants (hardcoded per harness contract).
B, S = 512, 1024
VOCAB = 100000
CHAR_LEN = 32
N_POS = 40
MAX_CANDS = 4000
CTX_RATE = 0.15
PRI_RATE = 1.0
N_CORES = 8

P = 128                      # partitions
BS_CORE = (B // N_CORES) * S     # tokens per core = 65536
F = BS_CORE // P             # free dim per partition = 512
N_CAND = N_POS * MAX_CANDS   # 160000
N_ROWS = N_CAND + VOCAB      # 260000
ROW = CHAR_LEN + 1           # 33 ints per row: [word_id, chars...]
GCHUNK = 128                 # tokens-per-partition per indirect-gather chunk
N_GC = F // GCHUNK           # gather chunks
ECHUNK = 128                 # cols per ap_gather chunk
N_EC = F // ECHUNK

i32 = mybir.dt.int32
i16 = mybir.dt.int16
u8 = mybir.dt.uint8
f32 = mybir.dt.float32
Alu = mybir.AluOpType

_CACHE = {}


def _build_nc():
    nc = bacc.Bacc(
        "TRN2", target_bir_lowering=False, debug=False, num_swdge_queues=4
    )

    word = nc.declare_dram_parameter("word", [P, F], i32, isOutput=False)
    pos = nc.declare_dram_parameter("pos", [P, F], i32, isOutput=False)
    msk = nc.declare_dram_parameter("msk", [P, F], i32, isOutput=False)
    ctx = nc.declare_dram_parameter("ctx", [P, F], f32, isOutput=False)
    pri = nc.declare_dram_parameter("pri", [P, F], f32, isOutput=False)
    cu = nc.declare_dram_parameter("cu", [P, F], f32, isOutput=False)
    btab = nc.declare_dram_parameter("btab", [N_ROWS, ROW], i32, isOutput=False)
    etab = nc.declare_dram_parameter("etab", [N_POS], i32, isOutput=False)
    m16 = nc.declare_dram_parameter("m16", [P, 16], i32, isOutput=False)

    o_word = nc.declare_dram_parameter("o_word", [P, F], i32, isOutput=True)
    o_char = nc.declare_dram_parameter("o_char", [P, F, CHAR_LEN], i32, isOutput=True)
    o_obf = nc.declare_dram_parameter("o_obf", [P, F], u8, isOutput=True)
    o_pri = nc.declare_dram_parameter("o_pri", [P, F], u8, isOutput=True)
    o_cpy = nc.declare_dram_parameter("o_cpy", [P, F], u8, isOutput=True)

    with tile.TileContext(nc) as tc:
        with (
            tc.tile_pool(name="singles", bufs=1) as singles,
            tc.tile_pool(name="inp", bufs=1) as inp,
            tc.tile_pool(name="ew", bufs=1) as ew,
            tc.tile_pool(name="exp", bufs=2) as expp,
            tc.tile_pool(name="grows", bufs=2) as growsp,
            tc.tile_pool(name="cc", bufs=2) as ccp,
        ):
            # --- constants ---
            etab_sb = singles.tile([P, N_POS], i32)
            eap = etab[:]
            nc.sync.dma_start(
                out=etab_sb[:, :],
                in_=bass.AP(
                    tensor=eap.tensor, offset=eap.offset, ap=[[0, P], eap.ap[0]]
                ),
            )
            m16_sb = singles.tile([P, 16], i32)
            nc.sync.dma_start(out=m16_sb[:, :], in_=m16[:, :])

            # --- load per-token inputs ---
            t_word = inp.tile([P, F], i32)
            t_pos = inp.tile([P, F], i32)
            t_msk = inp.tile([P, F], i32)
            t_ctx = inp.tile([P, F], f32)
            t_pri = inp.tile([P, F], f32)
            t_cu = inp.tile([P, F], f32)
            nc.sync.dma_start(out=t_word[:, :], in_=word[:, :])
            nc.sync.dma_start(out=t_pos[:, :], in_=pos[:, :])
            nc.sync.dma_start(out=t_msk[:, :], in_=msk[:, :])
            nc.sync.dma_start(out=t_ctx[:, :], in_=ctx[:, :])
            nc.sync.dma_start(out=t_pri[:, :], in_=pri[:, :])
            nc.sync.dma_start(out=t_cu[:, :], in_=cu[:, :])

            # --- 40-entry table lookup: E[p, j] = etab[pos[p, j]] ---
            pos16 = ew.tile([P, F], i16)
            nc.vector.tensor_copy(out=pos16[:, :], in_=t_pos[:, :])

            t_E = ew.tile([P, F], i32)
            for c in range(N_EC):
                cs = c * ECHUNK
                ce = cs + ECHUNK
                nidx = 16 * ECHUNK
                t_exp = expp.tile([P, nidx], i32)
                nc.gpsimd.ap_gather(
                    out_ap=t_exp[:, :],
                    in_ap=etab_sb[:, :],
                    idxs_ap=pos16[:, cs:ce],
                    channels=P,
                    num_elems=N_POS,
                    d=1,
                    num_idxs=nidx,
                )
                # mask out all but the per-partition diagonal, reduce inner 16
                exp3 = t_exp[:, :].rearrange("p (j r) -> p j r", r=16)
                m16b = bass.AP(
                    tensor=m16_sb.tensor,
                    offset=m16_sb[:, :].offset,
                    ap=[m16_sb[:, :].ap[0], [0, ECHUNK], m16_sb[:, :].ap[1]],
                )
                t_prod = expp.tile([P, nidx], i32, tag="prod")
                nc.vector.tensor_tensor(
                    out=t_prod[:, :].rearrange("p (j r) -> p j r", r=16),
                    in0=exp3,
                    in1=m16b,
                    op=Alu.mult,
                )
                with nc.allow_low_precision(
                    reason="masked one-hot sum of values < 2^15; exact in f32"
                ):
                    nc.vector.tensor_reduce(
                        out=t_E[:, cs:ce],
                        in_=t_prod[:, :].rearrange("p (j r) -> p j r", r=16),
                        axis=mybir.AxisListType.X,
                        op=Alu.add,
                    )

            # --- elementwise index math (full [P, F] tiles) ---
            # etab value E = priv*16384 + counts
            t_cnt = ew.tile([P, F], i32)
            nc.vector.tensor_scalar(
                out=t_cnt[:, :], in0=t_E[:, :], scalar1=16383, scalar2=None,
                op0=Alu.bitwise_and,
            )
            t_cntf = ew.tile([P, F], f32)
            nc.vector.tensor_copy(out=t_cntf[:, :], in_=t_cnt[:, :])
            t_cm1 = ew.tile([P, F], i32)
            nc.vector.tensor_scalar(
                out=t_cm1[:, :], in0=t_cnt[:, :], scalar1=-1, scalar2=None,
                op0=Alu.add,
            )
            t_pm = ew.tile([P, F], u8)
            nc.vector.tensor_scalar(
                out=t_pm[:, :], in0=t_E[:, :], scalar1=16383, scalar2=None,
                op0=Alu.is_gt,
            )

            t_t = ew.tile([P, F], f32)
            nc.vector.tensor_tensor(
                out=t_t[:, :], in0=t_cu[:, :], in1=t_cntf[:, :], op=Alu.mult
            )
            t_it = ew.tile([P, F], i32)
            nc.vector.tensor_copy(out=t_it[:, :], in_=t_t[:, :])
            t_tf = ew.tile([P, F], f32)
            nc.vector.tensor_copy(out=t_tf[:, :], in_=t_it[:, :])
            t_gt = ew.tile([P, F], i32)
            nc.vector.tensor_tensor(
                out=t_gt[:, :], in0=t_tf[:, :], in1=t_t[:, :], op=Alu.is_gt
            )
            t_it2 = ew.tile([P, F], i32)
            nc.vector.tensor_tensor(
                out=t_it2[:, :], in0=t_it[:, :], in1=t_gt[:, :], op=Alu.subtract
            )
            t_idx = ew.tile([P, F], i32)
            nc.vector.tensor_tensor(
                out=t_idx[:, :], in0=t_it2[:, :], in1=t_cm1[:, :], op=Alu.min
            )
            # gobf = pos*4000 + idx  (candidate row)
            t_gobf = ew.tile([P, F], i32)
            nc.vector.scalar_tensor_tensor(
                out=t_gobf[:, :], in0=t_pos[:, :], scalar=MAX_CANDS,
                in1=t_idx[:, :], op0=Alu.mult, op1=Alu.add,
            )
            # g = word + 160000 (identity row), overwritten where obf
            t_g = ew.tile([P, F], i32)
            nc.vector.tensor_scalar(
                out=t_g[:, :], in0=t_word[:, :], scalar1=N_CAND, scalar2=None,
                op0=Alu.add,
            )
            # obf mask
            t_a = ew.tile([P, F], u8)
            nc.vector.tensor_scalar(
                out=t_a[:, :], in0=t_pri[:, :], scalar1=PRI_RATE, scalar2=None,
                op0=Alu.is_lt,
            )
            t_obf = ew.tile([P, F], u8)
            nc.vector.tensor_scalar(
                out=t_obf[:, :], in0=t_ctx[:, :], scalar1=CTX_RATE, scalar2=None,
                op0=Alu.is_lt,
            )
            nc.vector.copy_predicated(
                out=t_obf[:, :], mask=t_pm[:, :], data=t_a[:, :]
            )
            nc.vector.copy_predicated(
                out=t_g[:, :], mask=t_obf[:, :], data=t_gobf[:, :]
            )

            # --- big gather: rows of 33 ints -> w column + char block ---
            # HW indirect DMA consumes exactly one offset per partition per
            # call (multi-index offset APs are broken in walrus), so gather
            # one 128-token column at a time.
            t_w = ew.tile([P, F], i32)
            for c in range(N_GC):
                cs = c * GCHUNK
                ce = cs + GCHUNK
                t_rows = growsp.tile([P, GCHUNK, ROW], i32)
                for k in range(GCHUNK):
                    inst = nc.gpsimd.indirect_dma_start(
                        out=t_rows[:, k, :],
                        out_offset=None,
                        in_=btab[:, :],
                        in_offset=bass.IndirectOffsetOnAxis(
                            ap=t_g[:, cs + k : cs + k + 1], axis=0
                        ),
                    )
                    # spread calls over the 4 SWDGE queues: each queue's
                    # descriptor ring serializes its own calls, so round-robin
                    # keeps 4 gathers in flight instead of 1.
                    qn = (c * GCHUNK + k) % 4
                    if qn:
                        inst.ins.queue = f"qPoolDynamic{qn}"
                # word-id column (stride ROW)
                nc.vector.tensor_copy(
                    out=t_w[:, cs:ce], in_=t_rows[:, :, 0]
                )
                # strip the id column: copy chars as packed int16 (DVE 4x)
                rows16 = t_rows[:, :, :].bitcast(i16)  # [P, GCHUNK, 2*ROW]
                t_cc = ccp.tile([P, GCHUNK * CHAR_LEN * 2], i16)
                nc.vector.tensor_copy(
                    out=t_cc[:, :].rearrange(
                        "p (j k) -> p j k", k=2 * CHAR_LEN
                    ),
                    in_=rows16[:, :, 2 : 2 + 2 * CHAR_LEN],
                )
                nc.sync.dma_start(
                    out=o_char[:, cs:ce, :],
                    in_=t_cc[:, :].bitcast(i32),
                )

            # --- epilogue masks + small outputs ---
            t_weq = ew.tile([P, F], u8)
            nc.vector.tensor_tensor(
                out=t_weq[:, :], in0=t_w[:, :], in1=t_word[:, :], op=Alu.is_equal
            )
            t_m8 = ew.tile([P, F], u8)
            nc.vector.tensor_copy(out=t_m8[:, :], in_=t_msk[:, :])
            t_cpy = ew.tile([P, F], u8)
            nc.vector.tensor_tensor(
                out=t_cpy[:, :], in0=t_weq[:, :], in1=t_m8[:, :],
                op=Alu.logical_and,
            )

            nc.sync.dma_start(out=o_word[:, :], in_=t_w[:, :])
            nc.sync.dma_start(out=o_obf[:, :], in_=t_obf[:, :])
            nc.sync.dma_start(out=o_pri[:, :], in_=t_pm[:, :])
            nc.sync.dma_start(out=o_cpy[:, :], in_=t_cpy[:, :])

    nc.finalize()
    return nc


def _build_tables(lut, tgt_table, counts, priv_pos):
    lut = np.asarray(lut, dtype=np.int32)
    tgt = np.asarray(tgt_table, dtype=np.int32)
    counts = np.asarray(counts, dtype=np.int32)
    priv = np.asarray(priv_pos).astype(bool)

    btab = np.empty((N_ROWS, ROW), dtype=np.int32)
    tflat = tgt.reshape(-1)
    btab[:N_CAND, 0] = tflat
    btab[:N_CAND, 1:] = lut[tflat]
    btab[N_CAND:, 0] = np.arange(VOCAB, dtype=np.int32)
    btab[N_CAND:, 1:] = lut

    etab = (counts + (priv.astype(np.int32) << 14)).astype(np.int32)

    m16 = (np.arange(16)[None, :] == (np.arange(P) % 16)[:, None]).astype(
        np.int32
    )
    return btab, etab, m16


def kernel(
    inp_word,
    inp_char,
    inp_pos,
    inp_mask,
    ctx_rand,
    pri_rand,
    cand_u,
    lut,
    tgt_table,
    counts,
    priv_pos,
):
    if "nc" not in _CACHE:
        _CACHE["nc"] = _build_nc()
    nc = _CACHE["nc"]

    btab, etab, m16 = _build_tables(lut, tgt_table, counts, priv_pos)

    inp_word = np.ascontiguousarray(np.asarray(inp_word, dtype=np.int32))
    inp_pos = np.ascontiguousarray(np.asarray(inp_pos, dtype=np.int32))
    inp_mask = np.ascontiguousarray(np.asarray(inp_mask, dtype=np.int32))
    ctx_rand = np.ascontiguousarray(np.asarray(ctx_rand, dtype=np.float32))
    pri_rand = np.ascontiguousarray(np.asarray(pri_rand, dtype=np.float32))
    cand_u = np.ascontiguousarray(np.asarray(cand_u, dtype=np.float32))

    bpc = B // N_CORES
    in_maps = []
    for i in range(N_CORES):
        sl = slice(i * bpc, (i + 1) * bpc)
        in_maps.append(
            {
                "word": inp_word[sl].reshape(P, F),
                "pos": inp_pos[sl].reshape(P, F),
                "msk": inp_mask[sl].reshape(P, F),
                "ctx": ctx_rand[sl].reshape(P, F),
                "pri": pri_rand[sl].reshape(P, F),
                "cu": cand_u[sl].reshape(P, F),
                "btab": btab,
                "etab": etab,
                "m16": m16,
            }
        )

    res = run_bass_kernel_spmd(nc, in_maps, core_ids=list(range(N_CORES)))
    _CACHE["last_result"] = res

    obf_word = np.empty((B, S), dtype=np.int32)
    obf_char = np.empty((B, S, CHAR_LEN), dtype=np.int32)
    obf_mask = np.empty((B, S), dtype=bool)
    pri_mask = np.empty((B, S), dtype=bool)
    cpy_mask = np.empty((B, S), dtype=bool)
    for i in range(N_CORES):
        sl = slice(i * bpc, (i + 1) * bpc)
        r = res.results[i]
        obf_word[sl] = r["o_word"].reshape(bpc, S)
        obf_char[sl] = r["o_char"].reshape(bpc, S, CHAR_LEN)
        obf_mask[sl] = r["o_obf"].reshape(bpc, S).astype(bool)
        pri_mask[sl] = r["o_pri"].reshape(bpc, S).astype(bool)
        cpy_mask[sl] = r["o_cpy"].reshape(bpc, S).astype(bool)

    return (obf_word, inp_word, obf_char, inp_pos, obf_mask, pri_mask, cpy_mask)
